# revision 19
# baseline (speedup 1.0000x reference)
"""Tensor-parallel GQA attention prefill (B=1, S=2048, D=4096, 32 q-heads /
8 kv-heads, RoPE, causal) for 8 Trainium2 NeuronCores.

Sharding: head-parallel. Core g owns q-heads 4g..4g+3 and kv-head g
(exact GQA group), computes Q/K/V projections for its heads, RoPE,
causal attention, and the partial output projection over its 512
contraction dims of wo. The host sums the 8 partial outputs.

Per-core kernel (Bass/Tile):
  phase 1  Q/K/V projections from a resident transposed activation.
           o-group (q3,K,V) runs FIRST (c-loop outer, paced by the xt
           DMA arrival), so K/V rope + V transposes complete mid-phase;
           o-group (q0,q1,q2) runs second with ol OUTER and c inner
           (xt fully resident by then), evicting + RoPE-ing each head
           as soon as its accumulation closes - the eviction/rope tail
           behind the last projection matmul shrinks to one head and
           the PE never idles long enough to drop the HAM throttle
           state. RoPE is rot = cos2*qt + sin2s*swap(qt) (DVE
           stream_shuffle; sin2s carries the (-1)^row signs); the
           softmax 1/sqrt(HD) is folded into the Q eviction.
  phase 2  attention computed transposed: scoresT[k,q]. k-tiles process
           in PAIRS whose scores land in the two banks of one
           [128, 1024] PSUM tile so a single exp activation covers both
           (the scalar engine's exp throughput gates attention). Blocks
           run h-order [3,0,1,2] so the first block's inputs (from the
           FIRST projection o-group) are ready at the phase boundary.
           Causality is structural: future k-tiles are skipped, the
           128-wide block diagonal gets its -60000 additive mask via a
           tiny accumulating matmul, and the farthest diagonal pair
           computes/exps only its live column spans (the dead 640 of
           1024 columns were pure PE + scalar-exp waste). Unnormalized
           attnV accumulates in PSUM; softmax denominators: first exp
           pair doubles as two fp16 DVE accumulation chains, later
           pairs add elementwise on the DVE, two accumulating
           ones-matmuls collapse the partition (k) axis.
  phase 3  output projection per 128-row chunk over 8 concurrent PSUM
           banks; the 8 PSUM evictions land in one wide [128, 4096]
           staging tile so each row chunk ships as ONE store DMA. The
           final chunk runs eb-outer with immediate eviction
           (scalar/DVE alternating) and quarter-stores as soon as each
           pair of banks lands, so the end-of-kernel drain isn't
           waiting on a deep eviction chain.

Matmuls run in bf16/fp16 with fp32 PSUM accumulation (fp32 matmul is
4x slower on TRN2's PE; fp8 DoubleRow is 2x faster but random-sign
contractions keep the full ~4% elementwise quantization error, over
the accuracy budget).

Hard-won scheduling facts baked in here: PE streams at ~0.42-0.51
ns/column (machine-dependent) with LDWEIGHTS hidden; DMA-trigger
instructions cost ~700ns on their issuing engine (so triggers are
load-balanced across engines and stores are batched); DMA chunk shapes
must keep large contiguous per-partition spans; PSUM has only 8 banks;
a PE idle gap >3us drops the HAM throttle state to half clock for
~10us, so phase boundaries must keep the PE fed.
"""

import sys

if "/opt/trn_rl_repo" not in sys.path:
    sys.path.insert(0, "/opt/trn_rl_repo")

from contextlib import ExitStack

import numpy as np
import ml_dtypes

import concourse.bass as bass
import concourse.tile as tile
from concourse import mybir, bacc

BF16 = mybir.dt.bfloat16
F16 = mybir.dt.float16
F32 = mybir.dt.float32
NBF = ml_dtypes.bfloat16

S = 2048
D = 4096
HD = 128
HQ = 4                      # q heads per core
N_CORES = 8
SCALE = 1.0 / float(np.sqrt(128.0))
NEG = -1e9


def build_nc(S=S, D=D, num_devices=N_CORES):
    NCT = D // 128          # contraction tiles over model dim
    NSB = S // 512          # 512-wide seq blocks
    NST = S // 128          # 128-wide seq tiles
    NO = HQ + 1             # rotated o-tiles: 4 q heads + 1 k head
    NOV = NO + 1            # + v head
    NEB = D // 512          # output-proj e blocks
    NJT = HQ                # contraction j-tiles in output proj

    nc = bacc.Bacc("TRN2", target_bir_lowering=False, debug=False,
                   num_devices=num_devices)
    xt_d = nc.dram_tensor("xt", [D, S], BF16, kind="ExternalInput")
    wt_d = nc.dram_tensor("wt", [2, 128, NCT, 384], BF16, kind="ExternalInput")
    wot_d = nc.dram_tensor("wot", [NJT, 128, D], BF16, kind="ExternalInput")
    cos2_d = nc.dram_tensor("cos2", [128, S], BF16, kind="ExternalInput")
    sin2_d = nc.dram_tensor("sin2", [128, S], BF16, kind="ExternalInput")
    id_d = nc.dram_tensor("ident", [128, 128], F16, kind="ExternalInput")
    mask_d = nc.dram_tensor("maskn", [128, 128], F16, kind="ExternalInput")
    out_d = nc.dram_tensor("out", [S, D], BF16, kind="ExternalOutput")

    with tile.TileContext(nc) as tc, ExitStack() as outer:
        const = outer.enter_context(tc.tile_pool(name="const", bufs=1))
        qkp = outer.enter_context(tc.tile_pool(name="qkrot", bufs=1))
        vp = outer.enter_context(tc.tile_pool(name="vnat", bufs=1))

        id_sb = const.tile([128, 128], F16)
        maskn_sb = const.tile([128, 128], F16)

        # Rotated Q,K in T-layout: o-tile-major [o*S + s]; o 0..3 = q heads,
        # o 4 = k head.
        qk_rot = qkp.tile([128, NO * S], BF16)
        # V natural layout, t-tile-major: v_nat[t_local, tt*128 + d]
        v_nat = vp.tile([128, S], F16)

        # ---------------- phase 1: projections + RoPE ----------------
        # og1 = (q3, K, V) runs FIRST (c-outer, DMA-paced); og0 =
        # (q0,q1,q2) second (ol-outer, xt resident, early evictions).
        with ExitStack() as ph1:
            xtp = ph1.enter_context(tc.tile_pool(name="xtp", bufs=1))
            csp = ph1.enter_context(tc.tile_pool(name="cossin", bufs=2))
            wst = ph1.enter_context(tc.tile_pool(name="wstream", bufs=1))
            vts = ph1.enter_context(tc.tile_pool(name="vtsb", bufs=1))
            qts = ph1.enter_context(tc.tile_pool(name="qtmp", bufs=2))
            rtm = ph1.enter_context(tc.tile_pool(name="ropetmp", bufs=2))

            def load_w(og, ranges, name):
                # one tile holds the 3 o-heads of this group:
                # w[p, c, ol*128+f]; chunks are contiguous c-ranges so the
                # DMA moves large descriptors.
                w = wst.tile([128, NCT, 384], BF16, tag="wsb", name=name)
                for c0, c1 in ranges:
                    nc.sync.dma_start(
                        out=w[:, c0:c1, :],
                        in_=wt_d[og, :, c0:c1, :])
                return w

            # og1 weights first on the sync queue: small first chunk so the
            # first matmul's stationary lands immediately (finer chunking
            # than this measured SLOWER - small per-partition descriptor
            # spans hurt DMA efficiency more than just-in-time helps).
            w_sb1 = load_w(1, [(0, 1), (1, 4)] +
                           [(q * 4, (q + 1) * 4) for q in range(1, 8)],
                           "wsb_1")
            # og0's first c-chunk prefetched into its own small tile during
            # og1 so the o-group switch has no weight-load bubble.
            w0p = ph1.enter_context(tc.tile_pool(name="w0pre", bufs=1))
            w0pre = w0p.tile([128, 4, 384], BF16)
            nc.sync.dma_start(out=w0pre, in_=wt_d[0, :, 0:4, :])

            # xt c-tiles in column halves: the og1-sbp0 pass only reads
            # columns [0, 1024), so ship all first-halves before any second
            # half - second halves aren't consumed until ~55us but on the
            # shared HBM they starve the first pass. scalar/gpsimd parity
            # only: routing some first-halves via sync parks them behind
            # the 3.4MB weight stream (measured a 28us stall). RoPE tables
            # + attention consts also go after the first halves.
            xt_sb = xtp.tile([128, NCT, S], BF16)
            hS = S // 2
            for c in range(NCT):
                eng = nc.scalar if c % 2 == 0 else nc.gpsimd
                if c < 2:
                    qS = hS // 2
                    for qf in range(2):
                        eng.dma_start(
                            out=xt_sb[:, c, qf * qS:(qf + 1) * qS],
                            in_=xt_d[c * 128:(c + 1) * 128,
                                     qf * qS:(qf + 1) * qS])
                else:
                    eng.dma_start(
                        out=xt_sb[:, c, 0:hS],
                        in_=xt_d[c * 128:(c + 1) * 128, 0:hS])
            cos_sb = csp.tile([128, S], BF16, tag="cs", name="cos_sb")
            sin_sb = csp.tile([128, S], BF16, tag="cs", name="sin_sb")
            nc.gpsimd.dma_start(out=cos_sb, in_=cos2_d[:])
            nc.gpsimd.dma_start(out=sin_sb, in_=sin2_d[:])
            nc.gpsimd.dma_start(out=id_sb, in_=id_d[:])
            nc.gpsimd.dma_start(out=maskn_sb, in_=mask_d[:])
            for c in range(NCT):
                eng = nc.sync if c % 2 == 0 else nc.gpsimd
                eng.dma_start(
                    out=xt_sb[:, c, hS:S],
                    in_=xt_d[c * 128:(c + 1) * 128, hS:S])
            vt_sb = vts.tile([128, S], F16)

            SWAP_MASK = [i ^ 1 for i in range(32)]

            def rope_evict(o, sb, ps, dve_evict=False):
                # RoPE: rot = cos2*qt + sin2s*swap(qt), where swap is the
                # partition pair-swap (DVE stream_shuffle) and sin2s carries
                # the (-1)^row signs - no tensor-engine J matmul.
                qt_sb = qts.tile([128, 512], BF16, tag="qt", name="qt_sb")
                # q heads fold in the softmax 1/sqrt(HD) so the attention
                # exp runs scale-free. The LAST head's eviction goes on the
                # DVE so the scalar queue is free for phase 2's first exps.
                if dve_evict:
                    nc.vector.tensor_scalar_mul(
                        qt_sb, ps, SCALE if o < HQ else 1.0)
                else:
                    nc.scalar.activation(out=qt_sb, in_=ps,
                                         func=mybir.ActivationFunctionType.Copy,
                                         scale=(SCALE if o < HQ else 1.0))
                sh = rtm.tile([128, 512], BF16, tag="sh", name="sh")
                nc.vector.stream_shuffle(sh, qt_sb, SWAP_MASK)
                t1 = rtm.tile([128, 512], F32, tag="rt", name="t1")
                nc.vector.tensor_mul(t1, qt_sb,
                                     cos_sb[:, sb * 512:(sb + 1) * 512])
                nc.vector.tensor_mul(sh, sh,
                                     sin_sb[:, sb * 512:(sb + 1) * 512])
                nc.vector.tensor_add(
                    qk_rot[:, o * S + sb * 512: o * S + sb * 512 + 512], t1, sh)

            # One PSUM pool for the whole projection phase: og1-sbp0 uses
            # all 6 slots at once (c-outer, DMA-paced); og1-sbp1 and og0
            # allocate 2-slot ol-groups from the SAME pool (3-deep
            # rotation), so there is no pool hand-off and no eviction-chain
            # stall at the o-group switch.
            pps = ph1.enter_context(
                tc.tile_pool(name="projps", bufs=6, space="PSUM"))
            jpp = ph1.enter_context(
                tc.tile_pool(name="jps", bufs=2, space="PSUM"))

            def v_transpose(trange):
                # V: T-layout -> natural via PE transpose
                for t in trange:
                    tp = jpp.tile([128, 128], F16, tag="jps")
                    nc.tensor.transpose(
                        tp, vt_sb[:, t * 128:(t + 1) * 128], id_sb)
                    nc.vector.tensor_copy(
                        v_nat[:, t * 128:(t + 1) * 128], tp)

            # ---- og1: (q3, K, V), c-loop outer (both sb-pairs are paced
            # by the xt DMA arrival - an ol-outer sweep here outruns the
            # second-half xt stream and stalls the PE) ----
            for sbp in range(2):
                psl = [pps.tile([128, 512], F32, tag="projps",
                                name=f"projps1_{sbp}_{i}")
                       for i in range(6)]
                for c in range(NCT):
                    for ol in range(3):
                        for sbl in range(2):
                            sb = sbp * 2 + sbl
                            nc.tensor.matmul(
                                psl[ol * 2 + sbl],
                                w_sb1[:, c, ol * 128:(ol + 1) * 128],
                                xt_sb[:, c, sb * 512:(sb + 1) * 512],
                                start=(c == 0), stop=(c == NCT - 1))
                for ol in range(3):
                    o = 3 + ol
                    for sbl in range(2):
                        sb = sbp * 2 + sbl
                        ps = psl[ol * 2 + sbl]
                        if o < NO:
                            rope_evict(o, sb, ps)
                        else:
                            nc.scalar.activation(
                                out=vt_sb[:, sb * 512:(sb + 1) * 512],
                                in_=ps,
                                func=mybir.ActivationFunctionType.Copy)
                v_transpose(range(sbp * 8, sbp * 8 + 8))

            # ---- og0: ol outer / c inner (xt fully resident by now); each
            # head evicts + ropes right after its accumulation closes ----
            w_sb0 = load_w(0, [(q * 4, (q + 1) * 4) for q in range(1, 8)],
                           "wsb_0")

            def ol_pass(ol, o, sbp, wtile, wpre, dve_evict=False):
                psl2 = [pps.tile([128, 512], F32, tag="projps",
                                 name=f"projps_{o}_{sbp}_{i}")
                        for i in range(2)]
                for c in range(NCT):
                    if wpre is not None and c < 4:
                        wsl = wpre[:, c, ol * 128:(ol + 1) * 128]
                    else:
                        wsl = wtile[:, c, ol * 128:(ol + 1) * 128]
                    for sbl in range(2):
                        sb = sbp * 2 + sbl
                        nc.tensor.matmul(
                            psl2[sbl], wsl,
                            xt_sb[:, c, sb * 512:(sb + 1) * 512],
                            start=(c == 0), stop=(c == NCT - 1))
                for sbl in range(2):
                    sb = sbp * 2 + sbl
                    if o < NO:
                        rope_evict(o, sb, psl2[sbl], dve_evict=dve_evict)
                    else:
                        nc.scalar.activation(
                            out=vt_sb[:, sb * 512:(sb + 1) * 512],
                            in_=psl2[sbl],
                            func=mybir.ActivationFunctionType.Copy)

            for sbp in range(2):
                for ol in range(3):
                    ol_pass(ol, ol, sbp, w_sb0, w0pre,
                            dve_evict=(sbp == 1 and ol == 2))

        # ---------------- phase 2: attention ----------------
        aotp = outer.enter_context(tc.tile_pool(name="aot", bufs=1))
        wotp = outer.enter_context(tc.tile_pool(name="wotsb", bufs=1))
        stg = outer.enter_context(tc.tile_pool(name="stage", bufs=2))
        # aot[d, j, s] = head j attention out (normalized), T-layout
        aot = aotp.tile([128, NJT, S], BF16)
        wot_sb = wotp.tile([128, NJT, D], BF16)

        # out-proj interleave state: row chunks whose aot rows are fully
        # finalized get their output-projection matmuls injected into
        # phase 2's PE slack (phase 2 is scalar/exp-bound, PE ~84% busy),
        # evicted on the DVE and stored from phase 2. oi_state maps
        # stc -> [stage_tile, next_eb]; finished stcs are skipped in ph3.
        oi_queue = []
        oi_state = {}
        interleaved = set()

        with ExitStack() as ph2:
            etp = ph2.enter_context(tc.tile_pool(name="expt", bufs=6))
            accp = ph2.enter_context(tc.tile_pool(name="accp", bufs=3))
            rbp = ph2.enter_context(tc.tile_pool(name="rbc", bufs=2))
            onesp = ph2.enter_context(tc.tile_pool(name="onesp", bufs=1))
            spsp = ph2.enter_context(tc.tile_pool(name="sps", bufs=2, space="PSUM"))
            outpp = ph2.enter_context(tc.tile_pool(name="outps", bufs=2, space="PSUM"))
            rpsp = ph2.enter_context(tc.tile_pool(name="rps", bufs=1, space="PSUM"))
            oip = ph2.enter_context(tc.tile_pool(name="oip", bufs=1, space="PSUM"))
            ones_sb = onesp.tile([128, 128], F16)
            nc.vector.memset(ones_sb, 1.0)

            def emit_unit():
                # one (stc, eb) out-projection unit: 4 accumulating matmuls
                # into the single oip bank, DVE eviction into the chunk's
                # staging tile, store when the chunk completes
                if not oi_queue:
                    return
                stc = oi_queue[0]
                if stc not in oi_state:
                    oi_state[stc] = [
                        stg.tile([128, D], BF16, tag="stage",
                                 name=f"stage_i{stc}"), 0]
                stage_t, eb = oi_state[stc]
                ps = oip.tile([128, 512], F32, tag="oi")
                for j in range(NJT):
                    nc.tensor.matmul(
                        ps,
                        aot[:, j, stc * 128:(stc + 1) * 128],
                        wot_sb[:, j, eb * 512:(eb + 1) * 512],
                        start=(j == 0), stop=(j == NJT - 1))
                nc.vector.tensor_copy(
                    stage_t[:, eb * 512:(eb + 1) * 512], ps)
                oi_state[stc][1] = eb + 1
                if eb + 1 == NEB:
                    nc.sync.dma_start(
                        out=out_d[stc * 128:(stc + 1) * 128, :],
                        in_=stage_t)
                    oi_queue.pop(0)
                    del oi_state[stc]
                    interleaved.add(stc)

            for j in range(NJT):
                for half in range(2):
                    hw_ = D // 2
                    nc.sync.dma_start(
                        out=wot_sb[:, j, half * hw_:(half + 1) * hw_],
                        in_=wot_d[j, :, half * hw_:(half + 1) * hw_])

            # k-tiles processed in PAIRS: scores for kt and kt+1 land in the
            # two banks of one [128, 1024] PSUM tile, ONE exp activation
            # covers both (the scalar engine's exp throughput gates
            # attention). The 1/sqrt(HD) scale is folded into the Q eviction
            # so the exp runs with scale=1. The first exp pair doubles as
            # the two DVE denominator accumulation chains; later pairs add
            # elementwise on the DVE, then two accumulating ones-matmuls
            # collapse the partition (k) axis. The farthest diagonal pair
            # (avs = 256/384) computes scores and exp on live spans only.
            def finalize(fin):
                # denominator collapse + normalization for a finished
                # h-block; deferred one pair into the NEXT block so these
                # PE/DVE ops never sit between a block's last attnV and
                # the next block's first scores (which stalls the exp
                # stream at every h boundary)
                facc, foutps, fh, fjq = fin
                rps = rpsp.tile([128, 512], F32, tag="rps")
                nc.tensor.matmul(rps, ones_sb, facc[:, 0:512],
                                 start=True, stop=False)
                nc.tensor.matmul(rps, ones_sb, facc[:, 512:1024],
                                 start=False, stop=True)
                rinv = rbp.tile([128, 512], F32, tag="rinv")
                nc.vector.reciprocal_approx_fast(out=rinv, in_=rps)
                nc.vector.tensor_mul(
                    aot[:, fh, fjq * 512:(fjq + 1) * 512], foutps, rinv)

            H_ORDER = [3, 0, 1, 2]
            OI_PACE = 5           # one oproj unit per 5 attention pairs
            pair_ctr = 0
            pending = None
            for jq in range(NSB):
                nk = 4 * jq + 4       # causal: k-tiles 0..4jq+3
                npair = nk // 2
                for h in H_ORDER:
                    outps = outpp.tile([128, 512], F32, tag="outps")
                    acc = None
                    for p in range(npair):
                        kts = [2 * p, 2 * p + 1]
                        avs = [max(kt - 4 * jq, 0) * 128 for kt in kts]
                        bpair = avs[0] >= 256   # farthest diagonal pair
                        sps = spsp.tile([128, 1024], F32, tag="sps")
                        for i, kt in enumerate(kts):
                            diag = kt - 4 * jq >= 0
                            if bpair:
                                # live columns only; dead span is never
                                # written nor read
                                nc.tensor.matmul(
                                    sps[:, i * 512 + avs[i]:(i + 1) * 512],
                                    qk_rot[:, HQ * S + kt * 128:
                                           HQ * S + (kt + 1) * 128],
                                    qk_rot[:, h * S + jq * 512 + avs[i]:
                                           h * S + jq * 512 + 512],
                                    start=True, stop=False,
                                    skip_group_check=True)
                                nc.tensor.matmul(
                                    sps[:, i * 512 + avs[i]:
                                        i * 512 + avs[i] + 128],
                                    maskn_sb, id_sb,
                                    start=False, stop=True,
                                    skip_group_check=True)
                            else:
                                # full-width scores (columns below the live
                                # range are computed-but-dead) so one exp
                                # can cover the whole pair; the causal mask
                                # lands via a tiny accumulating matmul
                                # (maskn^T @ I = mask block), not a DVE op.
                                nc.tensor.matmul(
                                    sps[:, i * 512:(i + 1) * 512],
                                    qk_rot[:, HQ * S + kt * 128:
                                           HQ * S + (kt + 1) * 128],
                                    qk_rot[:, h * S + jq * 512:
                                           h * S + jq * 512 + 512],
                                    start=True, stop=not diag,
                                    skip_group_check=True)
                                if diag:
                                    nc.tensor.matmul(
                                        sps[:, i * 512 + avs[i]:
                                            i * 512 + avs[i] + 128],
                                        maskn_sb, id_sb,
                                        start=False, stop=True,
                                        skip_group_check=True)
                        if p == 0:
                            et = accp.tile([128, 1024], F16, tag="acc",
                                           name="acc")
                            acc = et
                        else:
                            et = etp.tile([128, 1024], F16, tag="et")
                        if bpair:
                            # two live-span exps (640 of 1024 cols are
                            # dead here - pure scalar-engine waste)
                            for i in range(2):
                                nc.scalar.activation(
                                    out=et[:, i * 512 + avs[i]:(i + 1) * 512],
                                    in_=sps[:, i * 512 + avs[i]:(i + 1) * 512],
                                    func=mybir.ActivationFunctionType.Exp)
                        else:
                            # one exp covers both banks
                            nc.scalar.activation(
                                out=et, in_=sps,
                                func=mybir.ActivationFunctionType.Exp)
                        if p == 0 and avs[1] > 0:
                            # acc1's masked-off head columns must be zero
                            nc.vector.memset(acc[:, 512:512 + avs[1]], 0.0)
                        for i, kt in enumerate(kts):
                            nc.tensor.matmul(
                                outps[:, avs[i]:],
                                v_nat[:, kt * 128:(kt + 1) * 128],
                                et[:, i * 512 + avs[i]:(i + 1) * 512],
                                start=(kt == 0), stop=(kt == nk - 1))
                        if p > 0:
                            for i in range(2):
                                nc.vector.tensor_add(
                                    acc[:, i * 512 + avs[i]:(i + 1) * 512],
                                    acc[:, i * 512 + avs[i]:(i + 1) * 512],
                                    et[:, i * 512 + avs[i]:(i + 1) * 512])
                        if p == 0 and pending is not None:
                            fh = pending[2]
                            finalize(pending)
                            pending = None
                            if h == 3 and fh == H_ORDER[-1]:
                                # previous jq block fully finalized: its
                                # four row chunks become interleavable
                                for stc in range(4 * (jq - 1), 4 * jq):
                                    oi_queue.append(stc)
                        pair_ctr += 1
                        if pair_ctr % OI_PACE == 0:
                            emit_unit()
                    pending = (acc, outps, h, jq)
            finalize(pending)
            # last jq block finalized: push its chunks and emit a few units
            # on the oip bank so the PE stays fed across the psum-pool
            # hand-off into phase 3
            for stc in range(4 * (NSB - 1), NST):
                oi_queue.append(stc)
            for _ in range(6):
                emit_unit()

        # ---------------- phase 3: output projection ----------------
        with ExitStack() as ph3:
            opsp = ph3.enter_context(tc.tile_pool(name="ops", bufs=8, space="PSUM"))

            for stc in range(NST):
                if stc in interleaved:
                    continue
                last = stc == NST - 1
                if stc in oi_state:
                    # chunk partially emitted during phase 2: finish the
                    # remaining e-blocks into its existing staging tile
                    stage_t, eb0 = oi_state.pop(stc)
                    for eb in range(eb0, NEB):
                        ps = opsp.tile([128, 512], F32, tag="ops",
                                       name=f"ops_f{stc}_{eb}")
                        for j in range(NJT):
                            nc.tensor.matmul(
                                ps,
                                aot[:, j, stc * 128:(stc + 1) * 128],
                                wot_sb[:, j, eb * 512:(eb + 1) * 512],
                                start=(j == 0), stop=(j == NJT - 1))
                        if eb % 2 == 1:
                            nc.vector.tensor_copy(
                                stage_t[:, eb * 512:(eb + 1) * 512], ps)
                        else:
                            nc.scalar.activation(
                                out=stage_t[:, eb * 512:(eb + 1) * 512],
                                in_=ps,
                                func=mybir.ActivationFunctionType.Copy)
                    nc.sync.dma_start(
                        out=out_d[stc * 128:(stc + 1) * 128, :],
                        in_=stage_t)
                    continue
                stage = stg.tile([128, D], BF16, tag="stage", name="stage")
                if not last:
                    psl = [opsp.tile([128, 512], F32, tag="ops",
                                     name=f"ops_{stc}_{i}")
                           for i in range(NEB)]
                    for j in range(NJT):
                        for eb in range(NEB):
                            nc.tensor.matmul(
                                psl[eb],
                                aot[:, j, stc * 128:(stc + 1) * 128],
                                wot_sb[:, j, eb * 512:(eb + 1) * 512],
                                start=(j == 0), stop=(j == NJT - 1))
                    for eb in range(NEB):
                        # alternate evictions scalar/DVE (DVE is idle in
                        # phase 3) so bank recycling never waits on a deep
                        # single-engine eviction chain
                        if eb % 2 == 1:
                            nc.vector.tensor_copy(
                                stage[:, eb * 512:(eb + 1) * 512], psl[eb])
                        else:
                            nc.scalar.activation(
                                out=stage[:, eb * 512:(eb + 1) * 512],
                                in_=psl[eb],
                                func=mybir.ActivationFunctionType.Copy)
                    # one wide store per row chunk (sync DMA triggers
                    # ~700ns each)
                    nc.sync.dma_start(
                        out=out_d[stc * 128:(stc + 1) * 128, :],
                        in_=stage)
                else:
                    # final chunk: eb-outer with immediate eviction
                    # (alternating scalar/DVE) and quarter stores as soon
                    # as each bank pair lands, so the kernel-end drain has
                    # a shallow tail
                    for eb in range(NEB):
                        ps = opsp.tile([128, 512], F32, tag="ops",
                                       name=f"ops_{stc}_{eb}")
                        for j in range(NJT):
                            nc.tensor.matmul(
                                ps,
                                aot[:, j, stc * 128:(stc + 1) * 128],
                                wot_sb[:, j, eb * 512:(eb + 1) * 512],
                                start=(j == 0), stop=(j == NJT - 1))
                        if eb % 2 == 1:
                            nc.vector.tensor_copy(
                                stage[:, eb * 512:(eb + 1) * 512], ps)
                            sp = eb // 2
                            # quarter stores on alternating trigger queues
                            # (both idle here) so the final store issues as
                            # early as possible
                            seng = nc.gpsimd if sp % 2 == 0 else nc.sync
                            seng.dma_start(
                                out=out_d[stc * 128:(stc + 1) * 128,
                                          sp * 1024:(sp + 1) * 1024],
                                in_=stage[:, sp * 1024:(sp + 1) * 1024])
                        else:
                            nc.scalar.activation(
                                out=stage[:, eb * 512:(eb + 1) * 512],
                                in_=ps,
                                func=mybir.ActivationFunctionType.Copy)

    nc.compile()
    return nc


# ---------------------------------------------------------------------------
# host-side prep


def make_consts(cos, sin):
    """cos/sin: [S, 64] f32 -> replicated T-layout (sin carries the RoPE
    pair signs) + identity + natural-layout diag mask for the mask-matmul."""
    cos2 = np.repeat(np.ascontiguousarray(cos.T), 2, axis=0).astype(NBF)
    sin2 = np.repeat(np.ascontiguousarray(sin.T), 2, axis=0).astype(np.float32)
    sin2[0::2] *= -1.0          # rot[2p] = cos*q[2p] - sin*q[2p+1]
    sin2 = sin2.astype(NBF)
    ident = np.eye(128, dtype=np.float16)
    k_idx = np.arange(128)[:, None]
    q_idx = np.arange(128)[None, :]
    # maskn[q, k]: stationary for the diag mask-matmul (maskn^T @ I);
    # -60000 (fits fp16) is plenty: exp(scale * -6e4) == 0
    maskn = np.where(q_idx.T >= k_idx.T, 0.0, -60000.0).astype(np.float16)
    return cos2, sin2, ident, maskn


def prep_all(x, wq, wk, wv, wo, cos, sin, n_cores=N_CORES):
    NCT = D // 128
    x2 = np.asarray(x, np.float32).reshape(S, D)
    xt = np.ascontiguousarray(x2.T).astype(NBF)
    wq = np.asarray(wq, np.float32)
    wk = np.asarray(wk, np.float32)
    wv = np.asarray(wv, np.float32)
    wo = np.asarray(wo, np.float32)
    cos2, sin2, ident, maskn = make_consts(
        np.asarray(cos, np.float32), np.asarray(sin, np.float32))
    in_maps = []
    for g in range(n_cores):
        w_cat = np.concatenate(
            [wq[g * 512:(g + 1) * 512],
             wk[g * 128:(g + 1) * 128],
             wv[g * 128:(g + 1) * 128]], axis=0)          # [768, D]
        # wt[og, p, c, ol*128 + f] = w_cat[og*384 + ol*128 + f, c*128 + p]
        wt = np.ascontiguousarray(
            w_cat.reshape(2, 3, 128, NCT, 128).transpose(0, 4, 3, 1, 2)
        ).reshape(2, 128, NCT, 384).astype(NBF)
        wot = np.ascontiguousarray(
            wo[:, g * 512:(g + 1) * 512].T).reshape(4, 128, D).astype(NBF)
        in_maps.append({
            "xt": xt, "wt": wt, "wot": wot, "cos2": cos2, "sin2": sin2,
            "ident": ident, "maskn": maskn,
        })
    return in_maps


_NC_CACHE = None


def _get_nc():
    global _NC_CACHE
    if _NC_CACHE is None:
        _NC_CACHE = build_nc()
    return _NC_CACHE


def kernel(x, wq, wk, wv, wo, cos, sin, mask, start_pos):
    # mask is the standard causal mask (start_pos=0 prefill) — the kernel
    # applies causality structurally, so neither input is shipped.
    from concourse.bass_utils import run_bass_kernel_spmd

    nc = _get_nc()
    in_maps = prep_all(x, wq, wk, wv, wo, cos, sin)
    res = run_bass_kernel_spmd(nc, in_maps, core_ids=list(range(N_CORES)))
    acc = np.zeros((S, D), np.float32)
    for r in res.results:
        acc += r["out"].astype(np.float32)
    return acc.reshape(1, S, D)


# revision 21
# speedup vs baseline: 1.1900x; 1.1900x over previous
"""Tensor-parallel GQA attention prefill (B=1, S=2048, D=4096, 32 q-heads /
8 kv-heads, RoPE, causal) for 8 Trainium2 NeuronCores.

Sharding: head-parallel. Core g owns q-heads 4g..4g+3 and kv-head g
(exact GQA group), computes Q/K/V projections for its heads, RoPE,
causal attention, and the partial output projection over its 512
contraction dims of wo. The host sums the 8 partial outputs.

Per-core kernel (Bass/Tile):
  phase 1  Q/K/V projections from a resident transposed activation.
           o-group (q3,K,V) runs FIRST (c-loop outer, paced by the xt
           DMA arrival), so K/V rope + V transposes complete mid-phase;
           o-group (q0,q1,q2) runs second with ol OUTER and c inner
           (xt fully resident by then), evicting + RoPE-ing each head
           as soon as its accumulation closes - the eviction/rope tail
           behind the last projection matmul shrinks to one head and
           the PE never idles long enough to drop the HAM throttle
           state. RoPE is rot = cos2*qt + sin2s*swap(qt) (DVE
           stream_shuffle; sin2s carries the (-1)^row signs); the
           softmax 1/sqrt(HD) is folded into the Q eviction.
  phase 2  attention computed transposed: scoresT[k,q]. k-tiles process
           in PAIRS whose scores land in the two banks of one
           [128, 1024] PSUM tile so a single exp activation covers both
           (the scalar engine's exp throughput gates attention). Blocks
           run h-order [3,0,1,2] so the first block's inputs (from the
           FIRST projection o-group) are ready at the phase boundary.
           Causality is structural: future k-tiles are skipped, the
           128-wide block diagonal gets its -60000 additive mask via a
           tiny accumulating matmul, and the farthest diagonal pair
           computes/exps only its live column spans (the dead 640 of
           1024 columns were pure PE + scalar-exp waste). Unnormalized
           attnV accumulates in PSUM; softmax denominators: first exp
           pair doubles as two fp16 DVE accumulation chains, later
           pairs add elementwise on the DVE, two accumulating
           ones-matmuls collapse the partition (k) axis.
  phase 3  output projection per 128-row chunk over 8 concurrent PSUM
           banks; the 8 PSUM evictions land in one wide [128, 4096]
           staging tile so each row chunk ships as ONE store DMA. The
           final chunk runs eb-outer with immediate eviction
           (scalar/DVE alternating) and quarter-stores as soon as each
           pair of banks lands, so the end-of-kernel drain isn't
           waiting on a deep eviction chain.

Matmuls run in bf16/fp16 with fp32 PSUM accumulation (fp32 matmul is
4x slower on TRN2's PE; fp8 DoubleRow is 2x faster but random-sign
contractions keep the full ~4% elementwise quantization error, over
the accuracy budget).

Hard-won scheduling facts baked in here: PE streams at ~0.42-0.51
ns/column (machine-dependent) with LDWEIGHTS hidden; DMA-trigger
instructions cost ~700ns on their issuing engine (so triggers are
load-balanced across engines and stores are batched); DMA chunk shapes
must keep large contiguous per-partition spans; PSUM has only 8 banks;
a PE idle gap >3us drops the HAM throttle state to half clock for
~10us, so phase boundaries must keep the PE fed.
"""

import sys

if "/opt/trn_rl_repo" not in sys.path:
    sys.path.insert(0, "/opt/trn_rl_repo")

from contextlib import ExitStack

import numpy as np
import ml_dtypes

import concourse.bass as bass
import concourse.tile as tile
from concourse import mybir, bacc

BF16 = mybir.dt.bfloat16
F16 = mybir.dt.float16
F32 = mybir.dt.float32
NBF = ml_dtypes.bfloat16

S = 2048
D = 4096
HD = 128
HQ = 4                      # q heads per core
N_CORES = 8
SCALE = 1.0 / float(np.sqrt(128.0))
NEG = -1e9


def build_nc(S=S, D=D, num_devices=N_CORES):
    NCT = D // 128          # contraction tiles over model dim
    NSB = S // 512          # 512-wide seq blocks
    NST = S // 128          # 128-wide seq tiles
    NO = HQ + 1             # rotated o-tiles: 4 q heads + 1 k head
    NOV = NO + 1            # + v head
    NEB = D // 512          # output-proj e blocks
    NJT = HQ                # contraction j-tiles in output proj

    nc = bacc.Bacc("TRN2", target_bir_lowering=False, debug=False,
                   num_devices=num_devices)
    xt_d = nc.dram_tensor("xt", [D, S], BF16, kind="ExternalInput")
    wt_d = nc.dram_tensor("wt", [2, 128, NCT, 384], BF16, kind="ExternalInput")
    wot_d = nc.dram_tensor("wot", [NJT, 128, D], BF16, kind="ExternalInput")
    cos2_d = nc.dram_tensor("cos2", [128, S], BF16, kind="ExternalInput")
    sin2_d = nc.dram_tensor("sin2", [128, S], BF16, kind="ExternalInput")
    id_d = nc.dram_tensor("ident", [128, 128], F16, kind="ExternalInput")
    mask_d = nc.dram_tensor("maskn", [128, 128], F16, kind="ExternalInput")
    out_d = nc.dram_tensor("out", [S, D], BF16, kind="ExternalOutput")

    with tile.TileContext(nc) as tc, ExitStack() as outer:
        const = outer.enter_context(tc.tile_pool(name="const", bufs=1))
        qkp = outer.enter_context(tc.tile_pool(name="qkrot", bufs=1))
        vp = outer.enter_context(tc.tile_pool(name="vnat", bufs=1))

        id_sb = const.tile([128, 128], F16)
        maskn_sb = const.tile([128, 128], F16)

        # Rotated Q,K in T-layout: o-tile-major [o*S + s]; o 0..3 = q heads,
        # o 4 = k head.
        qk_rot = qkp.tile([128, NO * S], BF16)
        # V natural layout, t-tile-major: v_nat[t_local, tt*128 + d]
        v_nat = vp.tile([128, S], F16)

        # ---------------- phase 1: projections + RoPE ----------------
        # og1 = (q3, K, V) runs FIRST (c-outer, DMA-paced); og0 =
        # (q0,q1,q2) second (ol-outer, xt resident, early evictions).
        with ExitStack() as ph1:
            xtp = ph1.enter_context(tc.tile_pool(name="xtp", bufs=1))
            csp = ph1.enter_context(tc.tile_pool(name="cossin", bufs=2))
            wst = ph1.enter_context(tc.tile_pool(name="wstream", bufs=1))
            vts = ph1.enter_context(tc.tile_pool(name="vtsb", bufs=1))
            qts = ph1.enter_context(tc.tile_pool(name="qtmp", bufs=2))
            rtm = ph1.enter_context(tc.tile_pool(name="ropetmp", bufs=2))

            def load_w(og, ranges, name):
                # one tile holds the 3 o-heads of this group:
                # w[p, c, ol*128+f]; chunks are contiguous c-ranges so the
                # DMA moves large descriptors.
                w = wst.tile([128, NCT, 384], BF16, tag="wsb", name=name)
                for c0, c1 in ranges:
                    nc.sync.dma_start(
                        out=w[:, c0:c1, :],
                        in_=wt_d[og, :, c0:c1, :])
                return w

            # og1 weights first on the sync queue: small first chunk so the
            # first matmul's stationary lands immediately (finer chunking
            # than this measured SLOWER - small per-partition descriptor
            # spans hurt DMA efficiency more than just-in-time helps).
            w_sb1 = load_w(1, [(0, 1), (1, 4)] +
                           [(q * 4, (q + 1) * 4) for q in range(1, 8)],
                           "wsb_1")
            # og0's first c-chunk prefetched into its own small tile during
            # og1 so the o-group switch has no weight-load bubble.
            w0p = ph1.enter_context(tc.tile_pool(name="w0pre", bufs=1))
            w0pre = w0p.tile([128, 4, 384], BF16)
            nc.sync.dma_start(out=w0pre, in_=wt_d[0, :, 0:4, :])

            # xt c-tiles in column halves: the og1-sbp0 pass only reads
            # columns [0, 1024), so ship all first-halves before any second
            # half - second halves aren't consumed until ~55us but on the
            # shared HBM they starve the first pass. scalar/gpsimd parity
            # only: routing some first-halves via sync parks them behind
            # the 3.4MB weight stream (measured a 28us stall). RoPE tables
            # + attention consts also go after the first halves.
            xt_sb = xtp.tile([128, NCT, S], BF16)
            hS = S // 2
            for c in range(NCT):
                eng = nc.scalar if c % 2 == 0 else nc.gpsimd
                if c < 2:
                    qS = hS // 2
                    for qf in range(2):
                        eng.dma_start(
                            out=xt_sb[:, c, qf * qS:(qf + 1) * qS],
                            in_=xt_d[c * 128:(c + 1) * 128,
                                     qf * qS:(qf + 1) * qS])
                else:
                    eng.dma_start(
                        out=xt_sb[:, c, 0:hS],
                        in_=xt_d[c * 128:(c + 1) * 128, 0:hS])
            cos_sb = csp.tile([128, S], BF16, tag="cs", name="cos_sb")
            sin_sb = csp.tile([128, S], BF16, tag="cs", name="sin_sb")
            nc.gpsimd.dma_start(out=cos_sb, in_=cos2_d[:])
            nc.gpsimd.dma_start(out=sin_sb, in_=sin2_d[:])
            nc.gpsimd.dma_start(out=id_sb, in_=id_d[:])
            nc.gpsimd.dma_start(out=maskn_sb, in_=mask_d[:])
            for c in range(NCT):
                eng = nc.sync if c % 2 == 0 else nc.gpsimd
                eng.dma_start(
                    out=xt_sb[:, c, hS:S],
                    in_=xt_d[c * 128:(c + 1) * 128, hS:S])
            vt_sb = vts.tile([128, S], F16)

            SWAP_MASK = [i ^ 1 for i in range(32)]

            def rope_evict(o, sb, ps, dve_evict=False):
                # RoPE: rot = cos2*qt + sin2s*swap(qt), where swap is the
                # partition pair-swap (DVE stream_shuffle) and sin2s carries
                # the (-1)^row signs - no tensor-engine J matmul.
                qt_sb = qts.tile([128, 512], BF16, tag="qt", name="qt_sb")
                # q heads fold in the softmax 1/sqrt(HD) so the attention
                # exp runs scale-free. The LAST head's eviction goes on the
                # DVE so the scalar queue is free for phase 2's first exps.
                if dve_evict:
                    nc.vector.tensor_scalar_mul(
                        qt_sb, ps, SCALE if o < HQ else 1.0)
                else:
                    nc.scalar.activation(out=qt_sb, in_=ps,
                                         func=mybir.ActivationFunctionType.Copy,
                                         scale=(SCALE if o < HQ else 1.0))
                sh = rtm.tile([128, 512], BF16, tag="sh", name="sh")
                nc.vector.stream_shuffle(sh, qt_sb, SWAP_MASK)
                t1 = rtm.tile([128, 512], F32, tag="rt", name="t1")
                nc.vector.tensor_mul(t1, qt_sb,
                                     cos_sb[:, sb * 512:(sb + 1) * 512])
                nc.vector.tensor_mul(sh, sh,
                                     sin_sb[:, sb * 512:(sb + 1) * 512])
                nc.vector.tensor_add(
                    qk_rot[:, o * S + sb * 512: o * S + sb * 512 + 512], t1, sh)

            # One PSUM pool for the whole projection phase: og1-sbp0 uses
            # all 6 slots at once (c-outer, DMA-paced); og1-sbp1 and og0
            # allocate 2-slot ol-groups from the SAME pool (3-deep
            # rotation), so there is no pool hand-off and no eviction-chain
            # stall at the o-group switch.
            pps = ph1.enter_context(
                tc.tile_pool(name="projps", bufs=6, space="PSUM"))
            jpp = ph1.enter_context(
                tc.tile_pool(name="jps", bufs=2, space="PSUM"))

            def v_transpose(trange):
                # V: T-layout -> natural via PE transpose
                for t in trange:
                    tp = jpp.tile([128, 128], F16, tag="jps")
                    nc.tensor.transpose(
                        tp, vt_sb[:, t * 128:(t + 1) * 128], id_sb)
                    nc.vector.tensor_copy(
                        v_nat[:, t * 128:(t + 1) * 128], tp)

            # ---- og1: (q3, K, V), c-loop outer (both sb-pairs are paced
            # by the xt DMA arrival - an ol-outer sweep here outruns the
            # second-half xt stream and stalls the PE) ----
            for sbp in range(2):
                psl = [pps.tile([128, 512], F32, tag="projps",
                                name=f"projps1_{sbp}_{i}")
                       for i in range(6)]
                for c in range(NCT):
                    for ol in range(3):
                        for sbl in range(2):
                            sb = sbp * 2 + sbl
                            nc.tensor.matmul(
                                psl[ol * 2 + sbl],
                                w_sb1[:, c, ol * 128:(ol + 1) * 128],
                                xt_sb[:, c, sb * 512:(sb + 1) * 512],
                                start=(c == 0), stop=(c == NCT - 1))
                for ol in range(3):
                    o = 3 + ol
                    for sbl in range(2):
                        sb = sbp * 2 + sbl
                        ps = psl[ol * 2 + sbl]
                        if o < NO:
                            rope_evict(o, sb, ps)
                        else:
                            nc.scalar.activation(
                                out=vt_sb[:, sb * 512:(sb + 1) * 512],
                                in_=ps,
                                func=mybir.ActivationFunctionType.Copy)
                v_transpose(range(sbp * 8, sbp * 8 + 8))

            # ---- og0: ol outer / c inner (xt fully resident by now); each
            # head evicts + ropes right after its accumulation closes ----
            w_sb0 = load_w(0, [(q * 4, (q + 1) * 4) for q in range(1, 8)],
                           "wsb_0")

            def ol_pass(ol, o, sbp, wtile, wpre, dve_evict=False):
                psl2 = [pps.tile([128, 512], F32, tag="projps",
                                 name=f"projps_{o}_{sbp}_{i}")
                        for i in range(2)]
                for c in range(NCT):
                    if wpre is not None and c < 4:
                        wsl = wpre[:, c, ol * 128:(ol + 1) * 128]
                    else:
                        wsl = wtile[:, c, ol * 128:(ol + 1) * 128]
                    for sbl in range(2):
                        sb = sbp * 2 + sbl
                        nc.tensor.matmul(
                            psl2[sbl], wsl,
                            xt_sb[:, c, sb * 512:(sb + 1) * 512],
                            start=(c == 0), stop=(c == NCT - 1))
                for sbl in range(2):
                    sb = sbp * 2 + sbl
                    if o < NO:
                        rope_evict(o, sb, psl2[sbl], dve_evict=dve_evict)
                    else:
                        nc.scalar.activation(
                            out=vt_sb[:, sb * 512:(sb + 1) * 512],
                            in_=psl2[sbl],
                            func=mybir.ActivationFunctionType.Copy)

            for sbp in range(2):
                for ol in range(3):
                    ol_pass(ol, ol, sbp, w_sb0, w0pre,
                            dve_evict=(sbp == 1 and ol == 2))

        # ---------------- phase 2: attention ----------------
        aotp = outer.enter_context(tc.tile_pool(name="aot", bufs=1))
        wotp = outer.enter_context(tc.tile_pool(name="wotsb", bufs=1))
        stg = outer.enter_context(tc.tile_pool(name="stage", bufs=2))
        # aot[d, j, s] = head j attention out (normalized), T-layout
        aot = aotp.tile([128, NJT, S], BF16)
        wot_sb = wotp.tile([128, NJT, D], BF16)

        # out-proj interleave state: row chunks whose aot rows are fully
        # finalized get their output-projection matmuls injected into
        # phase 2's PE slack (phase 2 is scalar/exp-bound, PE ~84% busy),
        # evicted on the DVE and stored from phase 2. oi_state maps
        # stc -> [stage_tile, next_eb]; finished stcs are skipped in ph3.
        oi_queue = []
        oi_state = {}
        interleaved = set()

        with ExitStack() as ph2:
            etp = ph2.enter_context(tc.tile_pool(name="expt", bufs=6))
            accp = ph2.enter_context(tc.tile_pool(name="accp", bufs=3))
            rbp = ph2.enter_context(tc.tile_pool(name="rbc", bufs=2))
            onesp = ph2.enter_context(tc.tile_pool(name="onesp", bufs=1))
            spsp = ph2.enter_context(tc.tile_pool(name="sps", bufs=2, space="PSUM"))
            outpp = ph2.enter_context(tc.tile_pool(name="outps", bufs=2, space="PSUM"))
            rpsp = ph2.enter_context(tc.tile_pool(name="rps", bufs=1, space="PSUM"))
            oip = ph2.enter_context(tc.tile_pool(name="oip", bufs=1, space="PSUM"))
            ones_sb = onesp.tile([128, 128], F16)
            nc.vector.memset(ones_sb, 1.0)

            def emit_unit():
                # one (stc, eb) out-projection unit: 4 accumulating matmuls
                # into the single oip bank, DVE eviction into the chunk's
                # staging tile, store when the chunk completes
                if not oi_queue:
                    return
                stc = oi_queue[0]
                if stc not in oi_state:
                    oi_state[stc] = [
                        stg.tile([128, D], BF16, tag="stage",
                                 name=f"stage_i{stc}"), 0]
                stage_t, eb = oi_state[stc]
                ps = oip.tile([128, 512], F32, tag="oi")
                for j in range(NJT):
                    nc.tensor.matmul(
                        ps,
                        aot[:, j, stc * 128:(stc + 1) * 128],
                        wot_sb[:, j, eb * 512:(eb + 1) * 512],
                        start=(j == 0), stop=(j == NJT - 1))
                nc.vector.tensor_copy(
                    stage_t[:, eb * 512:(eb + 1) * 512], ps)
                oi_state[stc][1] = eb + 1
                if eb + 1 == NEB:
                    nc.sync.dma_start(
                        out=out_d[stc * 128:(stc + 1) * 128, :],
                        in_=stage_t)
                    oi_queue.pop(0)
                    del oi_state[stc]
                    interleaved.add(stc)

            for j in range(NJT):
                for half in range(2):
                    hw_ = D // 2
                    nc.sync.dma_start(
                        out=wot_sb[:, j, half * hw_:(half + 1) * hw_],
                        in_=wot_d[j, :, half * hw_:(half + 1) * hw_])

            # k-tiles processed in PAIRS: scores for kt and kt+1 land in the
            # two banks of one [128, 1024] PSUM tile, ONE exp activation
            # covers both (the scalar engine's exp throughput gates
            # attention). The 1/sqrt(HD) scale is folded into the Q eviction
            # so the exp runs with scale=1. The first exp pair doubles as
            # the two DVE denominator accumulation chains; later pairs add
            # elementwise on the DVE, then two accumulating ones-matmuls
            # collapse the partition (k) axis. The farthest diagonal pair
            # (avs = 256/384) computes scores and exp on live spans only.
            def finalize(fin):
                # denominator collapse + normalization for a finished
                # h-block; deferred one pair into the NEXT block so these
                # PE/DVE ops never sit between a block's last attnV and
                # the next block's first scores (which stalls the exp
                # stream at every h boundary)
                facc, foutps, fh, fjq = fin
                rps = rpsp.tile([128, 512], F32, tag="rps")
                nc.tensor.matmul(rps, ones_sb, facc[:, 0:512],
                                 start=True, stop=False)
                nc.tensor.matmul(rps, ones_sb, facc[:, 512:1024],
                                 start=False, stop=True)
                rinv = rbp.tile([128, 512], F32, tag="rinv")
                nc.vector.reciprocal_approx_fast(out=rinv, in_=rps)
                nc.vector.tensor_mul(
                    aot[:, fh, fjq * 512:(fjq + 1) * 512], foutps, rinv)

            H_ORDER = [3, 0, 1, 2]
            OI_PACE = 6           # one oproj unit per 6 attention pairs
            pair_ctr = 0
            pending = None
            for jq in range(NSB):
                nk = 4 * jq + 4       # causal: k-tiles 0..4jq+3
                npair = nk // 2
                for h in H_ORDER:
                    outps = outpp.tile([128, 512], F32, tag="outps")
                    acc = None
                    for p in range(npair):
                        kts = [2 * p, 2 * p + 1]
                        avs = [max(kt - 4 * jq, 0) * 128 for kt in kts]
                        bpair = avs[0] >= 256   # farthest diagonal pair
                        sps = spsp.tile([128, 1024], F32, tag="sps")
                        for i, kt in enumerate(kts):
                            diag = kt - 4 * jq >= 0
                            if bpair:
                                # live columns only; dead span is never
                                # written nor read
                                nc.tensor.matmul(
                                    sps[:, i * 512 + avs[i]:(i + 1) * 512],
                                    qk_rot[:, HQ * S + kt * 128:
                                           HQ * S + (kt + 1) * 128],
                                    qk_rot[:, h * S + jq * 512 + avs[i]:
                                           h * S + jq * 512 + 512],
                                    start=True, stop=False,
                                    skip_group_check=True)
                                nc.tensor.matmul(
                                    sps[:, i * 512 + avs[i]:
                                        i * 512 + avs[i] + 128],
                                    maskn_sb, id_sb,
                                    start=False, stop=True,
                                    skip_group_check=True)
                            else:
                                # full-width scores (columns below the live
                                # range are computed-but-dead) so one exp
                                # can cover the whole pair; the causal mask
                                # lands via a tiny accumulating matmul
                                # (maskn^T @ I = mask block), not a DVE op.
                                nc.tensor.matmul(
                                    sps[:, i * 512:(i + 1) * 512],
                                    qk_rot[:, HQ * S + kt * 128:
                                           HQ * S + (kt + 1) * 128],
                                    qk_rot[:, h * S + jq * 512:
                                           h * S + jq * 512 + 512],
                                    start=True, stop=not diag,
                                    skip_group_check=True)
                                if diag:
                                    nc.tensor.matmul(
                                        sps[:, i * 512 + avs[i]:
                                            i * 512 + avs[i] + 128],
                                        maskn_sb, id_sb,
                                        start=False, stop=True,
                                        skip_group_check=True)
                        if p == 0:
                            et = accp.tile([128, 1024], F16, tag="acc",
                                           name="acc")
                            acc = et
                        else:
                            et = etp.tile([128, 1024], F16, tag="et")
                        if bpair:
                            # two live-span exps (640 of 1024 cols are
                            # dead here - pure scalar-engine waste)
                            for i in range(2):
                                nc.scalar.activation(
                                    out=et[:, i * 512 + avs[i]:(i + 1) * 512],
                                    in_=sps[:, i * 512 + avs[i]:(i + 1) * 512],
                                    func=mybir.ActivationFunctionType.Exp)
                        else:
                            # one exp covers both banks
                            nc.scalar.activation(
                                out=et, in_=sps,
                                func=mybir.ActivationFunctionType.Exp)
                        if p == 0 and avs[1] > 0:
                            # acc1's masked-off head columns must be zero
                            nc.vector.memset(acc[:, 512:512 + avs[1]], 0.0)
                        for i, kt in enumerate(kts):
                            nc.tensor.matmul(
                                outps[:, avs[i]:],
                                v_nat[:, kt * 128:(kt + 1) * 128],
                                et[:, i * 512 + avs[i]:(i + 1) * 512],
                                start=(kt == 0), stop=(kt == nk - 1))
                        if p > 0:
                            for i in range(2):
                                nc.vector.tensor_add(
                                    acc[:, i * 512 + avs[i]:(i + 1) * 512],
                                    acc[:, i * 512 + avs[i]:(i + 1) * 512],
                                    et[:, i * 512 + avs[i]:(i + 1) * 512])
                        if p == 0 and pending is not None:
                            fh = pending[2]
                            finalize(pending)
                            pending = None
                            if h == 3 and fh == H_ORDER[-1]:
                                # previous jq block fully finalized: its
                                # four row chunks become interleavable
                                for stc in range(4 * (jq - 1), 4 * jq):
                                    oi_queue.append(stc)
                        pair_ctr += 1
                        if pair_ctr % OI_PACE == 0:
                            emit_unit()
                    pending = (acc, outps, h, jq)
            finalize(pending)
            # last jq block finalized: push its chunks and emit a few units
            # on the oip bank so the PE stays fed across the psum-pool
            # hand-off into phase 3
            for stc in range(4 * (NSB - 1), NST):
                oi_queue.append(stc)
            for _ in range(6):
                emit_unit()

        # ---------------- phase 3: output projection ----------------
        with ExitStack() as ph3:
            opsp = ph3.enter_context(tc.tile_pool(name="ops", bufs=8, space="PSUM"))

            for stc in range(NST):
                if stc in interleaved:
                    continue
                last = stc == NST - 1
                if stc in oi_state:
                    # chunk partially emitted during phase 2: finish the
                    # remaining e-blocks into its existing staging tile
                    stage_t, eb0 = oi_state.pop(stc)
                    for eb in range(eb0, NEB):
                        ps = opsp.tile([128, 512], F32, tag="ops",
                                       name=f"ops_f{stc}_{eb}")
                        for j in range(NJT):
                            nc.tensor.matmul(
                                ps,
                                aot[:, j, stc * 128:(stc + 1) * 128],
                                wot_sb[:, j, eb * 512:(eb + 1) * 512],
                                start=(j == 0), stop=(j == NJT - 1))
                        if eb % 2 == 1:
                            nc.vector.tensor_copy(
                                stage_t[:, eb * 512:(eb + 1) * 512], ps)
                        else:
                            nc.scalar.activation(
                                out=stage_t[:, eb * 512:(eb + 1) * 512],
                                in_=ps,
                                func=mybir.ActivationFunctionType.Copy)
                    nc.sync.dma_start(
                        out=out_d[stc * 128:(stc + 1) * 128, :],
                        in_=stage_t)
                    continue
                stage = stg.tile([128, D], BF16, tag="stage", name="stage")
                if not last:
                    psl = [opsp.tile([128, 512], F32, tag="ops",
                                     name=f"ops_{stc}_{i}")
                           for i in range(NEB)]
                    for j in range(NJT):
                        for eb in range(NEB):
                            nc.tensor.matmul(
                                psl[eb],
                                aot[:, j, stc * 128:(stc + 1) * 128],
                                wot_sb[:, j, eb * 512:(eb + 1) * 512],
                                start=(j == 0), stop=(j == NJT - 1))
                    for eb in range(NEB):
                        # alternate evictions scalar/DVE (DVE is idle in
                        # phase 3) so bank recycling never waits on a deep
                        # single-engine eviction chain
                        if eb % 2 == 1:
                            nc.vector.tensor_copy(
                                stage[:, eb * 512:(eb + 1) * 512], psl[eb])
                        else:
                            nc.scalar.activation(
                                out=stage[:, eb * 512:(eb + 1) * 512],
                                in_=psl[eb],
                                func=mybir.ActivationFunctionType.Copy)
                    # one wide store per row chunk (sync DMA triggers
                    # ~700ns each)
                    nc.sync.dma_start(
                        out=out_d[stc * 128:(stc + 1) * 128, :],
                        in_=stage)
                else:
                    # final chunk: eb-outer with immediate eviction
                    # (alternating scalar/DVE) and quarter stores as soon
                    # as each bank pair lands, so the kernel-end drain has
                    # a shallow tail
                    for eb in range(NEB):
                        ps = opsp.tile([128, 512], F32, tag="ops",
                                       name=f"ops_{stc}_{eb}")
                        for j in range(NJT):
                            nc.tensor.matmul(
                                ps,
                                aot[:, j, stc * 128:(stc + 1) * 128],
                                wot_sb[:, j, eb * 512:(eb + 1) * 512],
                                start=(j == 0), stop=(j == NJT - 1))
                        if eb % 2 == 1:
                            nc.vector.tensor_copy(
                                stage[:, eb * 512:(eb + 1) * 512], ps)
                        else:
                            nc.scalar.activation(
                                out=stage[:, eb * 512:(eb + 1) * 512],
                                in_=ps,
                                func=mybir.ActivationFunctionType.Copy)
                        # per-eb stores on alternating trigger queues (both
                        # idle here) so the final store issues as early as
                        # possible and the end-of-kernel drain has minimal
                        # outstanding DMA
                        seng = nc.gpsimd if eb % 2 == 0 else nc.sync
                        seng.dma_start(
                            out=out_d[stc * 128:(stc + 1) * 128,
                                      eb * 512:(eb + 1) * 512],
                            in_=stage[:, eb * 512:(eb + 1) * 512])

    nc.compile()
    return nc


# ---------------------------------------------------------------------------
# host-side prep


def make_consts(cos, sin):
    """cos/sin: [S, 64] f32 -> replicated T-layout (sin carries the RoPE
    pair signs) + identity + natural-layout diag mask for the mask-matmul."""
    cos2 = np.repeat(np.ascontiguousarray(cos.T), 2, axis=0).astype(NBF)
    sin2 = np.repeat(np.ascontiguousarray(sin.T), 2, axis=0).astype(np.float32)
    sin2[0::2] *= -1.0          # rot[2p] = cos*q[2p] - sin*q[2p+1]
    sin2 = sin2.astype(NBF)
    ident = np.eye(128, dtype=np.float16)
    k_idx = np.arange(128)[:, None]
    q_idx = np.arange(128)[None, :]
    # maskn[q, k]: stationary for the diag mask-matmul (maskn^T @ I);
    # -60000 (fits fp16) is plenty: exp(scale * -6e4) == 0
    maskn = np.where(q_idx.T >= k_idx.T, 0.0, -60000.0).astype(np.float16)
    return cos2, sin2, ident, maskn


def prep_all(x, wq, wk, wv, wo, cos, sin, n_cores=N_CORES):
    NCT = D // 128
    x2 = np.asarray(x, np.float32).reshape(S, D)
    xt = np.ascontiguousarray(x2.T).astype(NBF)
    wq = np.asarray(wq, np.float32)
    wk = np.asarray(wk, np.float32)
    wv = np.asarray(wv, np.float32)
    wo = np.asarray(wo, np.float32)
    cos2, sin2, ident, maskn = make_consts(
        np.asarray(cos, np.float32), np.asarray(sin, np.float32))
    in_maps = []
    for g in range(n_cores):
        w_cat = np.concatenate(
            [wq[g * 512:(g + 1) * 512],
             wk[g * 128:(g + 1) * 128],
             wv[g * 128:(g + 1) * 128]], axis=0)          # [768, D]
        # wt[og, p, c, ol*128 + f] = w_cat[og*384 + ol*128 + f, c*128 + p]
        wt = np.ascontiguousarray(
            w_cat.reshape(2, 3, 128, NCT, 128).transpose(0, 4, 3, 1, 2)
        ).reshape(2, 128, NCT, 384).astype(NBF)
        wot = np.ascontiguousarray(
            wo[:, g * 512:(g + 1) * 512].T).reshape(4, 128, D).astype(NBF)
        in_maps.append({
            "xt": xt, "wt": wt, "wot": wot, "cos2": cos2, "sin2": sin2,
            "ident": ident, "maskn": maskn,
        })
    return in_maps


_NC_CACHE = None


def _get_nc():
    global _NC_CACHE
    if _NC_CACHE is None:
        _NC_CACHE = build_nc()
    return _NC_CACHE


def kernel(x, wq, wk, wv, wo, cos, sin, mask, start_pos):
    # mask is the standard causal mask (start_pos=0 prefill) — the kernel
    # applies causality structurally, so neither input is shipped.
    from concourse.bass_utils import run_bass_kernel_spmd

    nc = _get_nc()
    in_maps = prep_all(x, wq, wk, wv, wo, cos, sin)
    res = run_bass_kernel_spmd(nc, in_maps, core_ids=list(range(N_CORES)))
    acc = np.zeros((S, D), np.float32)
    for r in res.results:
        acc += r["out"].astype(np.float32)
    return acc.reshape(1, S, D)
